# revision 2
# baseline (speedup 1.0000x reference)
"""Causal single-head attention (B=2, S=4096, D=1024) with RoPE on 8 TRN2 NeuronCores.

Sharding: per batch element, the 32 kv chunks (128 rows) are dealt round-robin
to 4 cores (chunk k -> core k%4). Every core runs an identical 32-slot program:
slot j computes partial causal attention of query chunk j (128 rows) against
the first sched[j] = 128*(j//4+1) rows of the core's gathered kv buffer. The
causal/ownership mask is synthesized ON DEVICE: only the last 128-col block of
a slot ever needs masking (earlier owned chunks are strictly below the
diagonal), and that block is sel_a*TRI + sel_b with per-core selectors sent as
a tiny [C,NQC,2] input (TRI built once with affine_select).
Cores return unnormalized partials (o_un in bf16, rowmax, rowsum); the host
merges the 4 partial softmaxes per query row and normalizes.

All matmuls run in bf16 with fp32 PSUM accumulation. Q/K output features are
permuted (evens-then-odds) on the host so RoPE operates on contiguous halves;
the permutation cancels in Q.K^T. x arrives host-transposed and tile-blocked
so no on-device transposes of x are needed. Weights arrive host-blocked
([p, dc, e]) so all input DMAs are contiguous per partition row.
"""

import sys

sys.path.insert(0, "/opt/trn_rl_repo")

import math
from contextlib import ExitStack

import ml_dtypes
import numpy as np

import concourse.bass as bass
import concourse.tile as tile
from concourse import bacc, mybir
from concourse.bass_utils import run_bass_kernel_spmd
from concourse.masks import make_causal_mask, make_identity

BF16 = mybir.dt.bfloat16
F32 = mybir.dt.float32
NPBF16 = ml_dtypes.bfloat16

B, S, D = 2, 4096, 1024
H = D // 2
C = 128                      # chunk rows
NQC = S // C                 # 32 query-chunk slots
NKVC = NQC // 4              # 8 kv chunks per core
NKV = NKVC * C               # 1024 resident kv rows per core
SCHED = [C * (j // 4 + 1) for j in range(NQC)]   # static kv window per slot
QG = 512                     # phase-B query group rows (4 slots)
NG = S // QG                 # 8 groups
SPG = QG // C                # slots per group
SCALE = 1.0 / math.sqrt(D)
NEG = -30000.0

_CACHE = {}


def _build():
    """Build + schedule the (core-uniform) Bass program once."""
    nc = bacc.Bacc("TRN2", target_bir_lowering=False, debug=False,
                   enable_asserts=False, num_devices=8)

    # host-blocked transposed x: xq_b[g, p, dc, s] = x[g*QG+s, dc*128+p]
    xq_b = nc.dram_tensor("xq_b", [NG, C, 8, QG], BF16, kind="ExternalInput").ap()
    # xkv_b[g, p, dc, s] = x[kvrows[g*128+s], dc*128+p]
    xkv_b = nc.dram_tensor("xkv_b", [NKVC, C, 8, C], BF16, kind="ExternalInput").ap()
    # host-blocked weights: w*_b[p, dc, e] = W^T[dc*128+p, e]
    wq_b = nc.dram_tensor("wq_b", [C, 8, D], BF16, kind="ExternalInput").ap()
    wk_b = nc.dram_tensor("wk_b", [C, 8, D], BF16, kind="ExternalInput").ap()
    wv_b = nc.dram_tensor("wv_b", [C, 8, D], BF16, kind="ExternalInput").ap()
    # csq_b[g, p, c, s]: c in 0..3 cos, 4..7 sin (transposed rope tables)
    csq_b = nc.dram_tensor("csq_b", [NG, C, 8, QG], BF16, kind="ExternalInput").ap()
    # natural rope tables for the gathered kv rows: cols 0:H cos, H:D sin
    cs_kv = nc.dram_tensor("cs_kv", [NKV, D], BF16, kind="ExternalInput").ap()
    # per-slot mask selectors: [:, j, 0]=1 iff diagonal block, [:, j, 1]=NEG iff dead
    sels = nc.dram_tensor("sels", [C, NQC, 2], F32, kind="ExternalInput").ap()

    o_un = nc.dram_tensor("o_un", [NG, C, SPG * D], BF16, kind="ExternalOutput").ap()
    stats = nc.dram_tensor("stats", [C, NQC, 2], F32, kind="ExternalOutput").ap()

    with tile.TileContext(nc) as tc, ExitStack() as ctx:
        const_p = ctx.enter_context(tc.tile_pool(name="const", bufs=1))
        w_p = ctx.enter_context(tc.tile_pool(name="weights", bufs=1))
        kvres_p = ctx.enter_context(tc.tile_pool(name="kvres", bufs=1))
        stats_p = ctx.enter_context(tc.tile_pool(name="stats", bufs=1))

        ident = const_p.tile([C, C], BF16)
        make_identity(nc, ident[:])
        tri = const_p.tile([C, C], F32)
        make_causal_mask(nc, tri[:], mask_val=NEG)
        sels_sb = const_p.tile([C, NQC, 2], F32)
        nc.sync.dma_start(sels_sb[:], sels)

        wq_sb = w_p.tile([C, 8, D], BF16, tag="wq")
        wk_sb = w_p.tile([C, 8, D], BF16, tag="wk")
        wv_sb = w_p.tile([C, 8, D], BF16, tag="wv")
        nc.sync.dma_start(wq_sb[:], wq_b)
        nc.sync.dma_start(wk_sb[:], wk_b)
        nc.sync.dma_start(wv_sb[:], wv_b)

        kt_sb = kvres_p.tile([C, 8, NKV], BF16, tag="kt")     # [p, dc, kvpos]
        v_sb = kvres_p.tile([C, NKVC, D], BF16, tag="v")      # [p, kvchunk, d]
        stats_sb = stats_p.tile([C, NQC, 2], F32, tag="st")

        with tc.tile_pool(name="a1", bufs=2) as a1_p, \
             tc.tile_pool(name="b", bufs=2) as b_p, \
             tc.tile_pool(name="bq", bufs=2) as bq_p, \
             tc.tile_pool(name="bs", bufs=2) as bs_p, \
             tc.tile_pool(name="mmps", bufs=2, space="PSUM") as mmps_p, \
             tc.tile_pool(name="accps", bufs=2, space="PSUM") as accps_p, \
             tc.tile_pool(name="tpps", bufs=2, space="PSUM") as tpps_p:

            def emit_a1_chunk(g):
                rows = slice(g * C, (g + 1) * C)
                xt_sb = a1_p.tile([C, 8, C], BF16, tag="xtkv", name=f"xtkv_{g}")
                nc.sync.dma_start(xt_sb[:], xkv_b[g])
                cskv_sb = a1_p.tile([C, D], BF16, tag="cskv", name=f"cskv_{g}")
                nc.sync.dma_start(cskv_sb[:], cs_kv[rows, :])

                k_ps = accps_p.tile([C, D], F32, tag="acc", name=f"kps_{g}")
                v_ps = accps_p.tile([C, D], F32, tag="acc", name=f"vps_{g}")
                for h in range(2):
                    cols = slice(h * 512, (h + 1) * 512)
                    for dc in range(8):
                        nc.tensor.matmul(k_ps[:, cols], xt_sb[:, dc, :],
                                         wk_sb[:, dc, cols],
                                         start=(dc == 0), stop=(dc == 7))
                    for dc in range(8):
                        nc.tensor.matmul(v_ps[:, cols], xt_sb[:, dc, :],
                                         wv_sb[:, dc, cols],
                                         start=(dc == 0), stop=(dc == 7))
                nc.scalar.copy(v_sb[:, g, :], v_ps[:])

                # rope K in natural layout (halves are real|imag after permutation)
                kr_sb = a1_p.tile([C, D], BF16, tag="kr", name=f"kr_{g}")
                t0 = a1_p.tile([C, H], BF16, tag="t0", name=f"kt0_{g}")
                t1 = a1_p.tile([C, H], BF16, tag="t1", name=f"kt1_{g}")
                re, im = k_ps[:, 0:H], k_ps[:, H:D]
                ckv, skv = cskv_sb[:, 0:H], cskv_sb[:, H:D]
                nc.vector.tensor_mul(t0[:], re, ckv)
                nc.vector.tensor_mul(t1[:], im, skv)
                nc.vector.tensor_sub(kr_sb[:, 0:H], t0[:], t1[:])
                nc.vector.tensor_mul(t0[:], re, skv)
                nc.vector.tensor_mul(t1[:], im, ckv)
                nc.vector.tensor_add(kr_sb[:, H:D], t0[:], t1[:])

                for dc in range(8):
                    tp = tpps_p.tile([C, 1024], BF16, tag="tp", name=f"ktp_{g}_{dc}")
                    nc.tensor.transpose(tp[:, 0:C], kr_sb[:, dc * C:(dc + 1) * C], ident[:])
                    nc.scalar.copy(kt_sb[:, dc, g * C:(g + 1) * C], tp[:, 0:C])

            def emit_b_group(g):
                xt_sb = b_p.tile([C, 8, QG], BF16, tag="xtq", name=f"xtq_{g}")
                nc.sync.dma_start(xt_sb[:], xq_b[g])
                cst_sb = b_p.tile([C, 8, QG], BF16, tag="cst", name=f"cst_{g}")
                nc.sync.dma_start(cst_sb[:], csq_b[g])

                qraw_sb = bq_p.tile([C, 8, QG], BF16, tag="qraw", name=f"qraw_{g}")
                for e in range(8):
                    qp = mmps_p.tile([C, 512], F32, tag="mm", name=f"qp_{g}_{e}")
                    for dc in range(8):
                        nc.tensor.matmul(qp[:, 0:QG], wq_sb[:, dc, e * C:(e + 1) * C],
                                         xt_sb[:, dc, :],
                                         start=(dc == 0), stop=(dc == 7))
                    nc.scalar.copy(qraw_sb[:, e, :], qp[:, 0:QG])

                qt_sb = bq_p.tile([C, 8, QG], BF16, tag="qt", name=f"qt_{g}")
                for ec in range(4):
                    cc, ss = cst_sb[:, ec, :], cst_sb[:, ec + 4, :]
                    re, im = qraw_sb[:, ec, :], qraw_sb[:, ec + 4, :]
                    t0 = b_p.tile([C, QG], BF16, tag="rt0", name=f"rt0_{g}_{ec}")
                    t1 = b_p.tile([C, QG], BF16, tag="rt1", name=f"rt1_{g}_{ec}")
                    nc.vector.tensor_mul(t0[:], re, cc)
                    nc.vector.tensor_mul(t1[:], im, ss)
                    nc.vector.tensor_sub(qt_sb[:, ec, :], t0[:], t1[:])
                    t2 = b_p.tile([C, QG], BF16, tag="rt2", name=f"rt2_{g}_{ec}")
                    t3 = b_p.tile([C, QG], BF16, tag="rt3", name=f"rt3_{g}_{ec}")
                    nc.vector.tensor_mul(t2[:], re, ss)
                    nc.vector.tensor_mul(t3[:], im, cc)
                    nc.vector.tensor_add(qt_sb[:, ec + 4, :], t2[:], t3[:])

                og_sb = b_p.tile([C, SPG * D], BF16, tag="og", name=f"og_{g}")
                for jj in range(SPG):
                    j = SPG * g + jj
                    W = SCHED[j]
                    qc = slice(jj * C, (jj + 1) * C)

                    # last-block mask: sel_a * TRI + sel_b (0 | TRI | all-NEG)
                    m_blk = bs_p.tile([C, C], F32, tag="mblk", name=f"m_{j}")
                    nc.vector.tensor_scalar(m_blk[:], tri[:],
                                            sels_sb[:, j, 0:1], sels_sb[:, j, 1:2],
                                            op0=mybir.AluOpType.mult,
                                            op1=mybir.AluOpType.add)
                    sc_sb = bs_p.tile([C, 1024], F32, tag="scores", name=f"sc_{j}")
                    rmax = bs_p.tile([C, 1], F32, tag="rmax", name=f"rmax_{j}")

                    ntile = (W + 511) // 512
                    for t in range(ntile):
                        wt = min(512, W - t * 512)
                        cols = slice(t * 512, t * 512 + wt)
                        s_ps = mmps_p.tile([C, 512], F32, tag="mm", name=f"sps_{j}_{t}")
                        for dc in range(8):
                            nc.tensor.matmul(s_ps[:, 0:wt], qt_sb[:, dc, qc],
                                             kt_sb[:, dc, cols],
                                             start=(dc == 0), stop=(dc == 7))
                        if t == ntile - 1:
                            if wt > C:
                                nc.scalar.copy(sc_sb[:, t * 512:t * 512 + wt - C],
                                               s_ps[:, 0:wt - C])
                            nc.vector.tensor_add(sc_sb[:, W - C:W],
                                                 s_ps[:, wt - C:wt], m_blk[:])
                        else:
                            nc.scalar.copy(sc_sb[:, cols], s_ps[:, 0:wt])

                    nc.vector.tensor_reduce(rmax[:], sc_sb[:, 0:W],
                                            axis=mybir.AxisListType.X,
                                            op=mybir.AluOpType.max)
                    negm = bs_p.tile([C, 1], F32, tag="negm", name=f"negm_{j}")
                    nc.scalar.mul(negm[:], rmax[:], -SCALE)
                    p_sb = bs_p.tile([C, 1024], BF16, tag="p", name=f"p_{j}")
                    lsum = bs_p.tile([C, 1], F32, tag="lsum", name=f"lsum_{j}")
                    nc.scalar.activation(p_sb[:, 0:W], sc_sb[:, 0:W],
                                         mybir.ActivationFunctionType.Exp,
                                         bias=negm[:], scale=SCALE,
                                         accum_out=lsum[:])
                    nc.scalar.copy(stats_sb[:, j, 0:1], negm[:])
                    nc.scalar.copy(stats_sb[:, j, 1:2], lsum[:])

                    o_ps = accps_p.tile([C, D], F32, tag="acc", name=f"ops_{j}")
                    nsub = W // C
                    for s0 in range(0, nsub, 2):
                        npair = min(2, nsub - s0)
                        ptp = tpps_p.tile([C, 1024], BF16, tag="tp", name=f"ptp_{j}_{s0}")
                        for u in range(npair):
                            nc.tensor.transpose(ptp[:, u * C:(u + 1) * C],
                                                p_sb[:, (s0 + u) * C:(s0 + u + 1) * C],
                                                ident[:])
                        pt_sb = b_p.tile([C, 2 * C], BF16, tag="pt", name=f"pt_{j}_{s0}")
                        nc.scalar.copy(pt_sb[:, 0:npair * C], ptp[:, 0:npair * C])
                        for u in range(npair):
                            sI = s0 + u
                            for h in range(2):
                                cols = slice(h * 512, (h + 1) * 512)
                                nc.tensor.matmul(o_ps[:, cols], pt_sb[:, u * C:(u + 1) * C],
                                                 v_sb[:, sI, cols],
                                                 start=(sI == 0), stop=(sI == nsub - 1))
                    nc.scalar.copy(og_sb[:, jj * D:(jj + 1) * D], o_ps[:])
                nc.sync.dma_start(o_un[g], og_sb[:])

            # interleaved emission: B group g needs kv chunks <= g
            emit_a1_chunk(0)
            emit_a1_chunk(1)
            for g in range(NG):
                emit_b_group(g)
                if g + 2 < NKVC:
                    emit_a1_chunk(g + 2)

        nc.sync.dma_start(stats, stats_sb[:])

    nc.compile()
    return nc


def _prep_inputs(x, w_q, w_k, w_v, freqs_cos, freqs_sin):
    """Host-side per-core input maps (numpy)."""
    perm = np.concatenate([np.arange(0, D, 2), np.arange(1, D, 2)])
    wqT = np.ascontiguousarray(w_q[perm, :].T.astype(NPBF16))
    wkT = np.ascontiguousarray(w_k[perm, :].T.astype(NPBF16))
    wvT = np.ascontiguousarray(w_v.T.astype(NPBF16))

    def blk(wt):  # [D, D] -> [p, dc, e]
        return np.ascontiguousarray(wt.reshape(8, C, D).transpose(1, 0, 2))

    wq_b, wk_b, wv_b = blk(wqT), blk(wkT), blk(wvT)
    cosq_t = freqs_cos.astype(NPBF16).reshape(NG, QG, 4, C).transpose(0, 3, 2, 1)
    sinq_t = freqs_sin.astype(NPBF16).reshape(NG, QG, 4, C).transpose(0, 3, 2, 1)
    csq_b = np.ascontiguousarray(np.concatenate([cosq_t, sinq_t], axis=2))

    in_maps = []
    for core in range(8):
        b, i = divmod(core, 4)
        kcs = np.arange(i, NQC, 4)
        kvrows = (kcs[:, None] * C + np.arange(C)[None, :]).reshape(-1)
        xb = np.asarray(x[b]).astype(NPBF16)
        xq_b = np.ascontiguousarray(
            xb.reshape(NG, QG, 8, C).transpose(0, 3, 2, 1))
        xkv_b = np.ascontiguousarray(
            xb[kvrows].reshape(NKVC, C, 8, C).transpose(0, 3, 2, 1))
        cs_kv = np.ascontiguousarray(np.concatenate(
            [freqs_cos[kvrows].astype(NPBF16), freqs_sin[kvrows].astype(NPBF16)],
            axis=1))
        j_arr = np.arange(NQC)
        sel_a = (j_arr % 4 == i).astype(np.float32)
        sel_b = np.where(j_arr % 4 < i, np.float32(NEG), np.float32(0.0))
        sels = np.ascontiguousarray(np.broadcast_to(
            np.stack([sel_a, sel_b], axis=1)[None], (C, NQC, 2)).astype(np.float32))
        in_maps.append({
            "xq_b": xq_b, "xkv_b": xkv_b,
            "wq_b": wq_b, "wk_b": wk_b, "wv_b": wv_b,
            "csq_b": csq_b, "cs_kv": cs_kv, "sels": sels,
        })
    return in_maps


def _merge(results):
    """Host softmax-merge of per-core partials -> [B,S,D] f32 (vectorized)."""
    out = np.empty((B, S, D), np.float32)
    for b in range(B):
        rs = [results[4 * b + i] for i in range(4)]
        mr = np.stack([-np.asarray(r["stats"], np.float32)[:, :, 0] for r in rs])
        ls = np.stack([np.asarray(r["stats"], np.float32)[:, :, 1] for r in rs])
        M = mr.max(axis=0)                          # [C, NQC]
        w = np.exp(mr - M[None])                    # [4, C, NQC]
        den = (w * ls).sum(axis=0)                  # [C, NQC]
        o = np.stack([np.asarray(r["o_un"], np.float32).reshape(NG, C, SPG, D)
                      for r in rs])                 # [4, NG, C, SPG, D]
        w4 = w.reshape(4, C, NG, SPG).transpose(0, 2, 1, 3)     # [4, NG, C, SPG]
        num = np.einsum('igcj,igcjd->gcjd', w4, o)
        den_g = den.reshape(C, NG, SPG).transpose(1, 0, 2)      # [NG, C, SPG]
        outb = num / den_g[..., None]
        out[b] = outb.transpose(0, 2, 1, 3).reshape(S, D)
    return out


def kernel(x, w_q, w_k, w_v, freqs_cos, freqs_sin, _want_results=False, _trace=False):
    if "nc" not in _CACHE:
        _CACHE["nc"] = _build()
    nc = _CACHE["nc"]
    in_maps = _prep_inputs(np.asarray(x, np.float32), np.asarray(w_q, np.float32),
                           np.asarray(w_k, np.float32), np.asarray(w_v, np.float32),
                           np.asarray(freqs_cos, np.float32),
                           np.asarray(freqs_sin, np.float32))
    kr = run_bass_kernel_spmd(nc, in_maps, core_ids=list(range(8)), trace=_trace)
    out = _merge(kr.results)
    if _want_results:
        return out, kr
    return out
